# revision 25
# baseline (speedup 1.0000x reference)
"""Trainium2 Bass kernel for nn_ContextEncoding (vq_codebook).

Computation (see reference):
  p = conv1x1(x) -> BN2d -> ReLU -> z [B,N,C]
  assign = softmax_k(scale_k * ||z - c_k||^2)
  agg[b,k,:] = sum_n assign[b,n,k] * (z[b,n,:] - c_k)
  e = relu(bn1d(agg)); feat = e.mean(k); gamma = sigmoid(fc(feat))
  out = relu(x * (1 + gamma[:, :, None, None]))

Sharding: 8 cores = (batch b in 0..3) x (spatial half h in 0..1).
Each core processes x[b, :, h*8192:(h+1)*8192] fully on-chip; the tiny
[K, C] aggregate is AllReduce'd between the two cores of each batch
pair; the final channel gate is applied from the SBUF-resident x copy.

kernel(**inputs) takes the FULL unsharded inputs and returns
(feat [B,C], out [B,C,H,W]) exactly like the reference.
"""

import os
import sys

import numpy as np

# ---- problem constants (hardcoded; kernel.py must be self-contained) ----
B, C, H, W, K = 4, 512, 128, 128, 32
N = H * W                    # 16384 spatial positions per batch
NSH = N // 2                 # 8192 positions per core
NCORES = 8
EPS = 1e-5

P = 128                      # partitions
CCH = C // P                 # 4 channel chunks
NSUP = NSH // 512            # 16 n-supers of 512
REPLICA_GROUPS = [[0, 1], [2, 3], [4, 5], [6, 7]]

TRACE = bool(int(os.environ.get("KERNEL_TRACE", "0")))
# debug bisection knobs
NO_CC = bool(int(os.environ.get("KERNEL_NO_CC", "0")))      # skip collective
# LVL: 1=conv only, 2=+transpose, 3=+softmax, 4=+agg/final (no cc), 5=full
LVL = int(os.environ.get("KERNEL_LVL", "5"))

_prog_cache = {}


def _import_concourse():
    try:
        import concourse.bass  # noqa: F401
    except ImportError:
        for p in ("/opt/trn_rl_repo", os.path.expanduser("~/.axon_site/_ro/trn_rl_repo")):
            if os.path.isdir(p) and p not in sys.path:
                sys.path.insert(0, p)
        import concourse.bass  # noqa: F401


def _emit(nc, tc, d):
    """Emit the per-core program. d: dict of DRAM tensor handles."""
    import concourse.bass as bass  # noqa: F401
    from concourse import mybir

    f32 = mybir.dt.float32
    Relu = mybir.ActivationFunctionType.Relu
    Exp = mybir.ActivationFunctionType.Exp
    Sigmoid = mybir.ActivationFunctionType.Sigmoid
    Square = mybir.ActivationFunctionType.Square
    add = mybir.AluOpType.add
    mult = mybir.AluOpType.mult
    subtract = mybir.AluOpType.subtract

    ctx = tc._emit_ctx  # ExitStack installed by caller

    # ---------------- pools ----------------
    const = ctx.enter_context(tc.tile_pool(name="const", bufs=1))
    zpool = ctx.enter_context(tc.tile_pool(name="z", bufs=2))
    zntp = ctx.enter_context(tc.tile_pool(name="znt", bufs=3))
    sqp = ctx.enter_context(tc.tile_pool(name="sq", bufs=2))
    smp = ctx.enter_context(tc.tile_pool(name="sm", bufs=3))
    statp = ctx.enter_context(tc.tile_pool(name="stat", bufs=3))
    outp = ctx.enter_context(tc.tile_pool(name="outs", bufs=4))
    finp = ctx.enter_context(tc.tile_pool(name="fin", bufs=1))

    pp = ctx.enter_context(tc.tile_pool(name="pp", bufs=2, space="PSUM"))
    pt = ctx.enter_context(tc.tile_pool(name="pt", bufs=2, space="PSUM"))
    px = ctx.enter_context(tc.tile_pool(name="px", bufs=2, space="PSUM"))
    pagg = ctx.enter_context(tc.tile_pool(name="pagg", bufs=1, space="PSUM"))
    pasum = ctx.enter_context(tc.tile_pool(name="pasum", bufs=1, space="PSUM"))

    dram = ctx.enter_context(tc.tile_pool(name="dram", bufs=1, space="DRAM"))

    # ---------------- constants into SBUF ----------------
    x_sb = []
    wT_sb = []
    cwT2_sb = []
    fcwT_sb = []
    for ic in range(CCH):
        xt = const.tile([P, NSH], f32, tag=f"x{ic}")
        x_sb.append(xt)
        wt = const.tile([P, C], f32, tag=f"wT{ic}")
        nc.sync.dma_start(wt[:], d["wT"][ic * P:(ic + 1) * P, :])
        wT_sb.append(wt)
        ct = const.tile([P, K], f32, tag=f"cwT2{ic}")
        nc.sync.dma_start(ct[:], d["cwT2"][ic * P:(ic + 1) * P, :])
        cwT2_sb.append(ct)
        ft = const.tile([P, C], f32, tag=f"fcwT{ic}")
        nc.sync.dma_start(ft[:], d["fcwT"][ic * P:(ic + 1) * P, :])
        fcwT_sb.append(ft)

    sc2b_sb = const.tile([P, K], f32, tag="sc2b")
    nc.sync.dma_start(sc2b_sb[:], d["sc2b"][:])
    scaleb_sb = const.tile([P, K], f32, tag="scaleb")
    nc.sync.dma_start(scaleb_sb[:], d["scaleb"][:])
    cw_sb = const.tile([K, C], f32, tag="cw")
    nc.sync.dma_start(cw_sb[:], d["cw"][:])
    s2_sb = const.tile([P, CCH], f32, tag="s2")
    nc.sync.dma_start(s2_sb[:], d["s2"][:])
    t2_sb = const.tile([P, CCH], f32, tag="t2")
    nc.sync.dma_start(t2_sb[:], d["t2"][:])
    fcb_sb = const.tile([P, CCH], f32, tag="fcb")
    nc.sync.dma_start(fcb_sb[:], d["fcb"][:])
    s1_sb = const.tile([K, 1], f32, tag="s1")
    nc.sync.dma_start(s1_sb[:], d["s1"][:])
    t1_sb = const.tile([K, 1], f32, tag="t1")
    nc.sync.dma_start(t1_sb[:], d["t1"][:])

    # identity / ones constants via DMA: keep gpsimd queue exclusively
    # for the collective (mixing pool ops + CC is a hang suspect)
    ident = const.tile([P, P], f32, tag="ident")
    nc.sync.dma_start(ident[:], d["ident"][:])
    ones_col = const.tile([P, 1], f32, tag="ones_col")
    nc.sync.dma_start(ones_col[:], d["ones_col"][:])

    # x shard resident in SBUF, loaded in [128, 512] slices for fine deps
    for ic in range(CCH):
        for ns in range(NSUP):
            nc.sync.dma_start(
                x_sb[ic][:, ns * 512:(ns + 1) * 512],
                d["x"][ic * P:(ic + 1) * P, ns * 512:(ns + 1) * 512],
            )

    agg_ps = pagg.tile([K, C], f32, tag="agg")
    asum_ps = pasum.tile([K, 1], f32, tag="asum")

    # ---------------- main loop ----------------
    for ns in range(NSUP):
        nsl = slice(ns * 512, (ns + 1) * 512)
        # conv1x1 + BN + ReLU -> z_cn [c_out, n] in SBUF (4 chunks)
        z_t = []
        for oc in range(CCH):
            pp_t = pp.tile([P, 512], f32, tag="pp")
            for ic in range(CCH):
                nc.tensor.matmul(
                    pp_t[:],
                    wT_sb[ic][:, oc * P:(oc + 1) * P],
                    x_sb[ic][:, nsl],
                    start=(ic == 0),
                    stop=(ic == CCH - 1),
                )
            zt = zpool.tile([P, 512], f32, tag=f"z{oc}")
            nc.scalar.activation(
                zt[:], pp_t[:], Relu,
                bias=t2_sb[:, oc:oc + 1], scale=s2_sb[:, oc:oc + 1],
            )
            z_t.append(zt)
            if LVL < 4:
                nc.sync.dma_start(d["out"][oc * P:(oc + 1) * P, nsl], zt[:])

        for nsub in range(4 if LVL >= 2 else 0):
            gi = ns * 4 + nsub
            nsub_sl = slice(nsub * P, (nsub + 1) * P)
            # transpose z -> z_nt [n, c]
            znt = zntp.tile([P, C], f32, tag="znt")
            for oc in range(CCH):
                pt_t = pt.tile([P, P], f32, tag="pt")
                nc.tensor.transpose(pt_t[:], z_t[oc][:, nsub_sl], ident[:])
                if oc % 2 == 0:
                    nc.vector.tensor_copy(znt[:, oc * P:(oc + 1) * P], pt_t[:])
                else:
                    nc.scalar.copy(znt[:, oc * P:(oc + 1) * P], pt_t[:])
            if LVL < 3:
                continue
            # x2 = sum_c z^2 (Act Square + accum; DVE ttr crashes trn2 HW)
            sq_t = sqp.tile([P, C], f32, tag="sq")
            x2c = statp.tile([P, 1], f32, tag="x2")
            nc.scalar.activation(sq_t[:], znt[:], Square, accum_out=x2c[:])
            # xc part: psum = -2*scale*xc  (scale folded into cwT2 on host)
            px_t = px.tile([P, K], f32, tag="px")
            for oc in range(CCH):
                nc.tensor.matmul(
                    px_t[:], z_t[oc][:, nsub_sl], cwT2_sb[oc][:],
                    start=(oc == 0), stop=(oc == CCH - 1),
                )
            # scaled_l2 = scale*x2 - 2*scale*xc + scale*c2; softmax over K
            # (no max-sub: scaled_l2 in [-450, -0.02] here, exp stays normal)
            d_t = smp.tile([P, K], f32, tag="d")
            nc.vector.tensor_scalar(d_t[:], scaleb_sb[:], x2c[:], None, mult)
            m1_t = smp.tile([P, K], f32, tag="m1")
            nc.vector.tensor_tensor(m1_t[:], px_t[:], sc2b_sb[:], add)
            m_t = smp.tile([P, K], f32, tag="m")
            nc.vector.tensor_tensor(m_t[:], d_t[:], m1_t[:], add)
            e_t = smp.tile([P, K], f32, tag="e")
            den = statp.tile([P, 1], f32, tag="den")
            nc.scalar.activation(e_t[:], m_t[:], Exp, accum_out=den[:])
            rec = statp.tile([P, 1], f32, tag="rec")
            nc.vector.reciprocal(rec[:], den[:])
            a_t = smp.tile([P, K], f32, tag="a")
            nc.vector.tensor_scalar(a_t[:], e_t[:], rec[:], None, mult)
            if LVL < 4:
                continue
            # aggregate: agg += assign^T @ z_nt ; asum += assign^T @ 1
            nc.tensor.matmul(agg_ps[:], a_t[:], znt[:],
                             start=(gi == 0), stop=(gi == NSUP * 4 - 1))
            nc.tensor.matmul(asum_ps[:], a_t[:], ones_col[:],
                             start=(gi == 0), stop=(gi == NSUP * 4 - 1))

    if LVL < 4:
        feat_sb = finp.tile([P, CCH], f32, tag="feat_sb")
        nc.vector.tensor_copy(feat_sb[:], s2_sb[:])
        nc.sync.dma_start(d["feat"][:], feat_sb[:])
        return

    # ---------------- aggregate correction + pair AllReduce ----------------
    asum_sb = finp.tile([K, 1], f32, tag="asum_sb")
    nc.vector.tensor_copy(asum_sb[:], asum_ps[:])
    tmp = finp.tile([K, C], f32, tag="tmp")
    nc.vector.tensor_scalar(tmp[:], cw_sb[:], asum_sb[:], None, mult)
    aggc = finp.tile([K, C], f32, tag="aggc")
    nc.vector.tensor_tensor(aggc[:], agg_ps[:], tmp[:], subtract)

    aggf = finp.tile([K, C], f32, tag="aggf")
    if NO_CC or LVL == 4:
        nc.vector.tensor_copy(aggf[:], aggc[:])
    else:
        ccin = dram.tile([K, C], f32, tag="ccin")
        ccout = dram.tile([K, C], f32, tag="ccout")
        nc.sync.dma_start(ccin[:], aggc[:])
        nc.gpsimd.collective_compute(
            "AllReduce", add, replica_groups=REPLICA_GROUPS,
            ins=[ccin[:].opt()], outs=[ccout[:].opt()],
        )
        nc.sync.dma_start(aggf[:], ccout[:])

    # ---------------- bn1 + relu + mean_k -> feat; fc -> gamma ----------------
    e_sb = finp.tile([K, C], f32, tag="e_sb")
    nc.scalar.activation(e_sb[:], aggf[:], Relu, bias=t1_sb[:], scale=s1_sb[:])
    pfeat = pt.tile([P, CCH], f32, tag="pt")
    for j in range(CCH):
        nc.tensor.matmul(pfeat[:, j:j + 1], e_sb[:, j * P:(j + 1) * P],
                         ones_col[0:K, :], start=True, stop=True)
    feat_sb = finp.tile([P, CCH], f32, tag="feat_sb")
    nc.scalar.mul(feat_sb[:], pfeat[:], 1.0 / K)
    nc.sync.dma_start(d["feat"][:], feat_sb[:])

    pgam = px.tile([P, CCH], f32, tag="px")
    for co in range(CCH):
        for ci in range(CCH):
            nc.tensor.matmul(
                pgam[:, co:co + 1],
                fcwT_sb[ci][:, co * P:(co + 1) * P],
                feat_sb[:, ci:ci + 1],
                start=(ci == 0), stop=(ci == CCH - 1),
            )
    g_sb = finp.tile([P, CCH], f32, tag="g_sb")
    for co in range(CCH):
        nc.scalar.activation(g_sb[:, co:co + 1], pgam[:, co:co + 1], Sigmoid,
                             bias=fcb_sb[:, co:co + 1])
    gp1 = finp.tile([P, CCH], f32, tag="gp1")
    nc.vector.tensor_scalar(gp1[:], g_sb[:], 1.0, None, add)

    # ---------------- gating: out = relu(x * (1 + gamma)) ----------------
    for oc in range(CCH):
        for ns in range(NSUP):
            nsl = slice(ns * 512, (ns + 1) * 512)
            o_t = outp.tile([P, 512], f32, tag="o")
            nc.scalar.activation(o_t[:], x_sb[oc][:, nsl], Relu,
                                 bias=0.0, scale=gp1[:, oc:oc + 1])
            nc.sync.dma_start(d["out"][oc * P:(oc + 1) * P, nsl], o_t[:])


def _build_program():
    if "nc" in _prog_cache:
        return _prog_cache["nc"]
    _import_concourse()
    from contextlib import ExitStack

    import concourse.tile as tile
    from concourse import bacc, mybir

    f32 = mybir.dt.float32
    nc = bacc.Bacc("TRN2", target_bir_lowering=False, debug=False,
                   num_devices=NCORES)

    d = {}
    d["x"] = nc.dram_tensor("x", [C, NSH], f32, kind="ExternalInput")
    d["wT"] = nc.dram_tensor("wT", [C, C], f32, kind="ExternalInput")
    d["cwT2"] = nc.dram_tensor("cwT2", [C, K], f32, kind="ExternalInput")
    d["sc2b"] = nc.dram_tensor("sc2b", [P, K], f32, kind="ExternalInput")
    d["scaleb"] = nc.dram_tensor("scaleb", [P, K], f32, kind="ExternalInput")
    d["cw"] = nc.dram_tensor("cw", [K, C], f32, kind="ExternalInput")
    d["s2"] = nc.dram_tensor("s2", [P, CCH], f32, kind="ExternalInput")
    d["t2"] = nc.dram_tensor("t2", [P, CCH], f32, kind="ExternalInput")
    d["s1"] = nc.dram_tensor("s1", [K, 1], f32, kind="ExternalInput")
    d["t1"] = nc.dram_tensor("t1", [K, 1], f32, kind="ExternalInput")
    d["fcwT"] = nc.dram_tensor("fcwT", [C, C], f32, kind="ExternalInput")
    d["fcb"] = nc.dram_tensor("fcb", [P, CCH], f32, kind="ExternalInput")
    d["ident"] = nc.dram_tensor("ident", [P, P], f32, kind="ExternalInput")
    d["ones_col"] = nc.dram_tensor("ones_col", [P, 1], f32, kind="ExternalInput")
    d["out"] = nc.dram_tensor("out", [C, NSH], f32, kind="ExternalOutput")
    d["feat"] = nc.dram_tensor("feat", [P, CCH], f32, kind="ExternalOutput")

    with tile.TileContext(nc) as tc:
        with ExitStack() as es:
            tc._emit_ctx = es
            _emit(nc, tc, d)
    nc.compile()
    _prog_cache["nc"] = nc
    return nc


def _host_prep(inputs):
    """Fold BN affines and pretranspose the small weights on the host."""
    f = np.float32
    conv_w = np.asarray(inputs["conv_w"], f)
    cw = np.asarray(inputs["codewords"], f)
    scale = np.asarray(inputs["scale"], f)
    fc_w = np.asarray(inputs["fc_w"], f)
    fc_b = np.asarray(inputs["fc_b"], f)

    s2 = (np.asarray(inputs["bn2_g"], np.float64)
          / np.sqrt(np.asarray(inputs["bn2_v"], np.float64) + EPS)).astype(f)
    t2 = (np.asarray(inputs["bn2_b"], np.float64)
          - np.asarray(inputs["bn2_m"], np.float64)
          * (np.asarray(inputs["bn2_g"], np.float64)
             / np.sqrt(np.asarray(inputs["bn2_v"], np.float64) + EPS))).astype(f)
    s1 = (np.asarray(inputs["bn1_g"], np.float64)
          / np.sqrt(np.asarray(inputs["bn1_v"], np.float64) + EPS)).astype(f)
    t1 = (np.asarray(inputs["bn1_b"], np.float64)
          - np.asarray(inputs["bn1_m"], np.float64)
          * (np.asarray(inputs["bn1_g"], np.float64)
             / np.sqrt(np.asarray(inputs["bn1_v"], np.float64) + EPS))).astype(f)

    c2 = (cw.astype(np.float64) ** 2).sum(axis=1).astype(f)

    common = {
        "wT": np.ascontiguousarray(conv_w.T),
        "cwT2": np.ascontiguousarray((-2.0 * cw * scale[:, None]).T.astype(f)),
        "sc2b": np.ascontiguousarray(
            np.broadcast_to((scale * c2).reshape(1, K), (P, K))),
        "scaleb": np.ascontiguousarray(np.broadcast_to(scale.reshape(1, K), (P, K))),
        "cw": np.ascontiguousarray(cw),
        "s2": np.ascontiguousarray(s2.reshape(CCH, P).T),
        "t2": np.ascontiguousarray(t2.reshape(CCH, P).T),
        "s1": np.ascontiguousarray(s1.reshape(K, 1)),
        "t1": np.ascontiguousarray(t1.reshape(K, 1)),
        "fcwT": np.ascontiguousarray(fc_w.T),
        "fcb": np.ascontiguousarray(fc_b.reshape(CCH, P).T),
        "ident": np.eye(P, dtype=f),
        "ones_col": np.ones((P, 1), f),
    }

    x = np.asarray(inputs["x"], f).reshape(B, C, N)
    in_maps = []
    for core in range(NCORES):
        b, h = core // 2, core % 2
        m = dict(common)
        m["x"] = np.ascontiguousarray(x[b, :, h * NSH:(h + 1) * NSH])
        in_maps.append(m)
    return in_maps


def kernel(**inputs):
    _import_concourse()
    from concourse.bass_utils import run_bass_kernel_spmd

    nc = _build_program()
    in_maps = _host_prep(inputs)
    res = run_bass_kernel_spmd(nc, in_maps, list(range(NCORES)), trace=TRACE)
    _prog_cache["last_results"] = res

    out = np.empty((B, C, N), np.float32)
    feat = np.empty((B, C), np.float32)
    for core in range(NCORES):
        b, h = core // 2, core % 2
        out[b, :, h * NSH:(h + 1) * NSH] = res.results[core]["out"]
        if h == 0:
            feat[b] = res.results[core]["feat"].T.reshape(C)
    return feat, out.reshape(B, C, H, W)


# revision 31
# speedup vs baseline: 1.4620x; 1.4620x over previous
"""Trainium2 Bass kernel for nn_ContextEncoding (vq_codebook).

Computation (see reference):
  p = conv1x1(x) -> BN2d -> ReLU -> z [B,N,C]
  assign = softmax_k(scale_k * ||z - c_k||^2)
  agg[b,k,:] = sum_n assign[b,n,k] * (z[b,n,:] - c_k)
  e = relu(bn1d(agg)); feat = e.mean(k); gamma = sigmoid(fc(feat))
  out = relu(x * (1 + gamma[:, :, None, None]))

Sharding: 8 cores = (batch b in 0..3) x (spatial half h in 0..1).
Each core processes x[b, :, h*8192:(h+1)*8192] fully on-chip; the tiny
[K, C] aggregate is AllReduce'd between the two cores of each batch
pair; the final channel gate is applied from the SBUF-resident x copy.

kernel(**inputs) takes the FULL unsharded inputs and returns
(feat [B,C], out [B,C,H,W]) exactly like the reference.
"""

import os
import sys

import numpy as np

# ---- problem constants (hardcoded; kernel.py must be self-contained) ----
B, C, H, W, K = 4, 512, 128, 128, 32
N = H * W                    # 16384 spatial positions per batch
NSH = N // 2                 # 8192 positions per core
NCORES = 8
EPS = 1e-5

P = 128                      # partitions
CCH = C // P                 # 4 channel chunks
NSUP = NSH // 512            # 16 n-supers of 512
REPLICA_GROUPS = [[0, 1], [2, 3], [4, 5], [6, 7]]

TRACE = bool(int(os.environ.get("KERNEL_TRACE", "0")))
# debug bisection knobs
NO_CC = bool(int(os.environ.get("KERNEL_NO_CC", "0")))      # skip collective
# LVL: 1=conv only, 2=+transpose, 3=+softmax, 4=+agg/final (no cc), 5=full
LVL = int(os.environ.get("KERNEL_LVL", "5"))

_prog_cache = {}


def _import_concourse():
    try:
        import concourse.bass  # noqa: F401
    except ImportError:
        for p in ("/opt/trn_rl_repo", os.path.expanduser("~/.axon_site/_ro/trn_rl_repo")):
            if os.path.isdir(p) and p not in sys.path:
                sys.path.insert(0, p)
        import concourse.bass  # noqa: F401


def _emit(nc, tc, d):
    """Emit the per-core program. d: dict of DRAM tensor handles."""
    import concourse.bass as bass  # noqa: F401
    from concourse import mybir

    f32 = mybir.dt.float32
    bf16 = mybir.dt.bfloat16
    Relu = mybir.ActivationFunctionType.Relu
    Exp = mybir.ActivationFunctionType.Exp
    Sigmoid = mybir.ActivationFunctionType.Sigmoid
    Square = mybir.ActivationFunctionType.Square
    add = mybir.AluOpType.add
    mult = mybir.AluOpType.mult
    subtract = mybir.AluOpType.subtract

    ctx = tc._emit_ctx  # ExitStack installed by caller

    # ---------------- pools ----------------
    const = ctx.enter_context(tc.tile_pool(name="const", bufs=1))
    zpool = ctx.enter_context(tc.tile_pool(name="z", bufs=2))
    zntp = ctx.enter_context(tc.tile_pool(name="znt", bufs=3))
    sqp = ctx.enter_context(tc.tile_pool(name="sq", bufs=2))
    smp = ctx.enter_context(tc.tile_pool(name="sm", bufs=3))
    statp = ctx.enter_context(tc.tile_pool(name="stat", bufs=3))
    outp = ctx.enter_context(tc.tile_pool(name="outs", bufs=4))
    finp = ctx.enter_context(tc.tile_pool(name="fin", bufs=1))

    pp = ctx.enter_context(tc.tile_pool(name="pp", bufs=2, space="PSUM"))
    pt = ctx.enter_context(tc.tile_pool(name="pt", bufs=2, space="PSUM"))
    px = ctx.enter_context(tc.tile_pool(name="px", bufs=2, space="PSUM"))
    pagg = ctx.enter_context(tc.tile_pool(name="pagg", bufs=1, space="PSUM"))
    pasum = ctx.enter_context(tc.tile_pool(name="pasum", bufs=1, space="PSUM"))

    dram = ctx.enter_context(tc.tile_pool(name="dram", bufs=1, space="DRAM"))

    # ---------------- constants into SBUF ----------------
    x_sb = []
    wT_sb = []
    cwT2_sb = []
    fcwT_sb = []
    for ic in range(CCH):
        xt = const.tile([P, NSH], bf16, tag=f"x{ic}")
        x_sb.append(xt)
        wt = const.tile([P, C], bf16, tag=f"wT{ic}")
        nc.sync.dma_start(wt[:], d["wT"][ic * P:(ic + 1) * P, :])
        wT_sb.append(wt)
        ct = const.tile([P, K], f32, tag=f"cwT2{ic}")
        nc.sync.dma_start(ct[:], d["cwT2"][ic * P:(ic + 1) * P, :])
        cwT2_sb.append(ct)
        ft = const.tile([P, C], f32, tag=f"fcwT{ic}")
        nc.sync.dma_start(ft[:], d["fcwT"][ic * P:(ic + 1) * P, :])
        fcwT_sb.append(ft)

    sc2b_sb = const.tile([P, K], f32, tag="sc2b")
    nc.sync.dma_start(sc2b_sb[:], d["sc2b"][:])
    scaleb_sb = const.tile([P, K], f32, tag="scaleb")
    nc.sync.dma_start(scaleb_sb[:], d["scaleb"][:])
    cw_sb = const.tile([K, C], f32, tag="cw")
    nc.sync.dma_start(cw_sb[:], d["cw"][:])
    s2_sb = const.tile([P, CCH], f32, tag="s2")
    nc.sync.dma_start(s2_sb[:], d["s2"][:])
    t2_sb = const.tile([P, CCH], f32, tag="t2")
    nc.sync.dma_start(t2_sb[:], d["t2"][:])
    fcb_sb = const.tile([P, CCH], f32, tag="fcb")
    nc.sync.dma_start(fcb_sb[:], d["fcb"][:])
    s1_sb = const.tile([K, 1], f32, tag="s1")
    nc.sync.dma_start(s1_sb[:], d["s1"][:])
    t1_sb = const.tile([K, 1], f32, tag="t1")
    nc.sync.dma_start(t1_sb[:], d["t1"][:])

    # identity / ones constants via DMA: keep gpsimd queue exclusively
    # for the collective (mixing pool ops + CC is a hang suspect)
    ident = const.tile([P, P], f32, tag="ident")
    nc.sync.dma_start(ident[:], d["ident"][:])
    ones_col = const.tile([P, 1], f32, tag="ones_col")
    nc.sync.dma_start(ones_col[:], d["ones_col"][:])

    # x shard resident in SBUF, loaded in [128, 512] slices for fine deps
    for ic in range(CCH):
        for ns in range(NSUP):
            nc.sync.dma_start(
                x_sb[ic][:, ns * 512:(ns + 1) * 512],
                d["x"][ic * P:(ic + 1) * P, ns * 512:(ns + 1) * 512],
            )

    agg_ps = pagg.tile([K, C], f32, tag="agg")
    asum_ps = pasum.tile([K, 1], f32, tag="asum")

    # ---------------- main loop ----------------
    for ns in range(NSUP):
        nsl = slice(ns * 512, (ns + 1) * 512)
        # conv1x1 + BN + ReLU -> z_cn [c_out, n] in SBUF (4 chunks)
        z_t = []
        for oc in range(CCH):
            pp_t = pp.tile([P, 512], f32, tag="pp")
            for ic in range(CCH):
                nc.tensor.matmul(
                    pp_t[:],
                    wT_sb[ic][:, oc * P:(oc + 1) * P],
                    x_sb[ic][:, nsl],
                    start=(ic == 0),
                    stop=(ic == CCH - 1),
                )
            zt = zpool.tile([P, 512], f32, tag=f"z{oc}")
            nc.scalar.activation(
                zt[:], pp_t[:], Relu,
                bias=t2_sb[:, oc:oc + 1], scale=s2_sb[:, oc:oc + 1],
            )
            z_t.append(zt)
            if LVL < 4:
                nc.sync.dma_start(d["out"][oc * P:(oc + 1) * P, nsl], zt[:])

        for nsub in range(4 if LVL >= 2 else 0):
            gi = ns * 4 + nsub
            nsub_sl = slice(nsub * P, (nsub + 1) * P)
            # transpose z -> z_nt [n, c]
            znt = zntp.tile([P, C], f32, tag="znt")
            for oc in range(CCH):
                pt_t = pt.tile([P, P], f32, tag="pt")
                nc.tensor.transpose(pt_t[:], z_t[oc][:, nsub_sl], ident[:])
                if oc % 2 == 0:
                    nc.vector.tensor_copy(znt[:, oc * P:(oc + 1) * P], pt_t[:])
                else:
                    nc.scalar.copy(znt[:, oc * P:(oc + 1) * P], pt_t[:])
            if LVL < 3:
                continue
            # x2 = sum_c z^2 (Act Square + accum; DVE ttr crashes trn2 HW)
            sq_t = sqp.tile([P, C], f32, tag="sq")
            x2c = statp.tile([P, 1], f32, tag="x2")
            nc.scalar.activation(sq_t[:], znt[:], Square, accum_out=x2c[:])
            # xc part: psum = -2*scale*xc  (scale folded into cwT2 on host)
            px_t = px.tile([P, K], f32, tag="px")
            for oc in range(CCH):
                nc.tensor.matmul(
                    px_t[:], z_t[oc][:, nsub_sl], cwT2_sb[oc][:],
                    start=(oc == 0), stop=(oc == CCH - 1),
                )
            # scaled_l2 = scale*x2 - 2*scale*xc + scale*c2; softmax over K
            # (no max-sub: scaled_l2 in [-450, -0.02] here, exp stays normal)
            d_t = smp.tile([P, K], f32, tag="d")
            nc.vector.tensor_scalar(d_t[:], scaleb_sb[:], x2c[:], None, mult)
            m1_t = smp.tile([P, K], f32, tag="m1")
            nc.vector.tensor_tensor(m1_t[:], px_t[:], sc2b_sb[:], add)
            m_t = smp.tile([P, K], f32, tag="m")
            nc.vector.tensor_tensor(m_t[:], d_t[:], m1_t[:], add)
            e_t = smp.tile([P, K], f32, tag="e")
            den = statp.tile([P, 1], f32, tag="den")
            nc.scalar.activation(e_t[:], m_t[:], Exp, accum_out=den[:])
            rec = statp.tile([P, 1], f32, tag="rec")
            nc.vector.reciprocal(rec[:], den[:])
            a_t = smp.tile([P, K], f32, tag="a")
            nc.vector.tensor_scalar(a_t[:], e_t[:], rec[:], None, mult)
            if LVL < 4:
                continue
            # aggregate: agg += assign^T @ z_nt ; asum += assign^T @ 1
            nc.tensor.matmul(agg_ps[:], a_t[:], znt[:],
                             start=(gi == 0), stop=(gi == NSUP * 4 - 1))
            nc.tensor.matmul(asum_ps[:], a_t[:], ones_col[:],
                             start=(gi == 0), stop=(gi == NSUP * 4 - 1))

    if LVL < 4:
        feat_sb = finp.tile([P, CCH], f32, tag="feat_sb")
        nc.vector.tensor_copy(feat_sb[:], s2_sb[:])
        nc.sync.dma_start(d["feat"][:], feat_sb[:])
        return

    # ---------------- aggregate correction + pair AllReduce ----------------
    asum_sb = finp.tile([K, 1], f32, tag="asum_sb")
    nc.vector.tensor_copy(asum_sb[:], asum_ps[:])
    tmp = finp.tile([K, C], f32, tag="tmp")
    nc.vector.tensor_scalar(tmp[:], cw_sb[:], asum_sb[:], None, mult)
    aggc = finp.tile([K, C], f32, tag="aggc")
    nc.vector.tensor_tensor(aggc[:], agg_ps[:], tmp[:], subtract)

    aggf = finp.tile([K, C], f32, tag="aggf")
    if NO_CC or LVL == 4:
        nc.vector.tensor_copy(aggf[:], aggc[:])
    else:
        ccin = dram.tile([K, C], f32, tag="ccin")
        ccout = dram.tile([K, C], f32, tag="ccout")
        nc.sync.dma_start(ccin[:], aggc[:])
        nc.gpsimd.collective_compute(
            "AllReduce", add, replica_groups=REPLICA_GROUPS,
            ins=[ccin[:].opt()], outs=[ccout[:].opt()],
        )
        nc.sync.dma_start(aggf[:], ccout[:])

    # ---------------- bn1 + relu + mean_k -> feat; fc -> gamma ----------------
    e_sb = finp.tile([K, C], f32, tag="e_sb")
    nc.scalar.activation(e_sb[:], aggf[:], Relu, bias=t1_sb[:], scale=s1_sb[:])
    pfeat = pt.tile([P, CCH], f32, tag="pt")
    for j in range(CCH):
        nc.tensor.matmul(pfeat[:, j:j + 1], e_sb[:, j * P:(j + 1) * P],
                         ones_col[0:K, :], start=True, stop=True)
    feat_sb = finp.tile([P, CCH], f32, tag="feat_sb")
    nc.scalar.mul(feat_sb[:], pfeat[:], 1.0 / K)
    nc.sync.dma_start(d["feat"][:], feat_sb[:])

    pgam = px.tile([P, CCH], f32, tag="px")
    for co in range(CCH):
        for ci in range(CCH):
            nc.tensor.matmul(
                pgam[:, co:co + 1],
                fcwT_sb[ci][:, co * P:(co + 1) * P],
                feat_sb[:, ci:ci + 1],
                start=(ci == 0), stop=(ci == CCH - 1),
            )
    g_sb = finp.tile([P, CCH], f32, tag="g_sb")
    for co in range(CCH):
        nc.scalar.activation(g_sb[:, co:co + 1], pgam[:, co:co + 1], Sigmoid,
                             bias=fcb_sb[:, co:co + 1])
    gp1 = finp.tile([P, CCH], f32, tag="gp1")
    nc.vector.tensor_scalar(gp1[:], g_sb[:], 1.0, None, add)

    # ---------------- gating: out = relu(x * (1 + gamma)) ----------------
    for oc in range(CCH):
        for ns in range(NSUP):
            nsl = slice(ns * 512, (ns + 1) * 512)
            o_t = outp.tile([P, 512], f32, tag="o")
            nc.scalar.activation(o_t[:], x_sb[oc][:, nsl], Relu,
                                 bias=0.0, scale=gp1[:, oc:oc + 1])
            nc.sync.dma_start(d["out"][oc * P:(oc + 1) * P, nsl], o_t[:])


def _build_program():
    if "nc" in _prog_cache:
        return _prog_cache["nc"]
    _import_concourse()
    from contextlib import ExitStack

    import concourse.tile as tile
    from concourse import bacc, mybir

    f32 = mybir.dt.float32
    nc = bacc.Bacc("TRN2", target_bir_lowering=False, debug=False,
                   num_devices=NCORES)

    d = {}
    bf16 = mybir.dt.bfloat16
    d["x"] = nc.dram_tensor("x", [C, NSH], bf16, kind="ExternalInput")
    d["wT"] = nc.dram_tensor("wT", [C, C], bf16, kind="ExternalInput")
    d["cwT2"] = nc.dram_tensor("cwT2", [C, K], f32, kind="ExternalInput")
    d["sc2b"] = nc.dram_tensor("sc2b", [P, K], f32, kind="ExternalInput")
    d["scaleb"] = nc.dram_tensor("scaleb", [P, K], f32, kind="ExternalInput")
    d["cw"] = nc.dram_tensor("cw", [K, C], f32, kind="ExternalInput")
    d["s2"] = nc.dram_tensor("s2", [P, CCH], f32, kind="ExternalInput")
    d["t2"] = nc.dram_tensor("t2", [P, CCH], f32, kind="ExternalInput")
    d["s1"] = nc.dram_tensor("s1", [K, 1], f32, kind="ExternalInput")
    d["t1"] = nc.dram_tensor("t1", [K, 1], f32, kind="ExternalInput")
    d["fcwT"] = nc.dram_tensor("fcwT", [C, C], f32, kind="ExternalInput")
    d["fcb"] = nc.dram_tensor("fcb", [P, CCH], f32, kind="ExternalInput")
    d["ident"] = nc.dram_tensor("ident", [P, P], f32, kind="ExternalInput")
    d["ones_col"] = nc.dram_tensor("ones_col", [P, 1], f32, kind="ExternalInput")
    d["out"] = nc.dram_tensor("out", [C, NSH], f32, kind="ExternalOutput")
    d["feat"] = nc.dram_tensor("feat", [P, CCH], f32, kind="ExternalOutput")

    with tile.TileContext(nc) as tc:
        with ExitStack() as es:
            tc._emit_ctx = es
            _emit(nc, tc, d)
    nc.compile()
    _prog_cache["nc"] = nc
    return nc


def _host_prep(inputs):
    """Fold BN affines and pretranspose the small weights on the host."""
    import ml_dtypes
    bf = ml_dtypes.bfloat16
    f = np.float32
    conv_w = np.asarray(inputs["conv_w"], f)
    cw = np.asarray(inputs["codewords"], f)
    scale = np.asarray(inputs["scale"], f)
    fc_w = np.asarray(inputs["fc_w"], f)
    fc_b = np.asarray(inputs["fc_b"], f)

    s2 = (np.asarray(inputs["bn2_g"], np.float64)
          / np.sqrt(np.asarray(inputs["bn2_v"], np.float64) + EPS)).astype(f)
    t2 = (np.asarray(inputs["bn2_b"], np.float64)
          - np.asarray(inputs["bn2_m"], np.float64)
          * (np.asarray(inputs["bn2_g"], np.float64)
             / np.sqrt(np.asarray(inputs["bn2_v"], np.float64) + EPS))).astype(f)
    s1 = (np.asarray(inputs["bn1_g"], np.float64)
          / np.sqrt(np.asarray(inputs["bn1_v"], np.float64) + EPS)).astype(f)
    t1 = (np.asarray(inputs["bn1_b"], np.float64)
          - np.asarray(inputs["bn1_m"], np.float64)
          * (np.asarray(inputs["bn1_g"], np.float64)
             / np.sqrt(np.asarray(inputs["bn1_v"], np.float64) + EPS))).astype(f)

    c2 = (cw.astype(np.float64) ** 2).sum(axis=1).astype(f)

    common = {
        "wT": np.ascontiguousarray(conv_w.T).astype(bf),
        "cwT2": np.ascontiguousarray((-2.0 * cw * scale[:, None]).T.astype(f)),
        "sc2b": np.ascontiguousarray(
            np.broadcast_to((scale * c2).reshape(1, K), (P, K))),
        "scaleb": np.ascontiguousarray(np.broadcast_to(scale.reshape(1, K), (P, K))),
        "cw": np.ascontiguousarray(cw),
        "s2": np.ascontiguousarray(s2.reshape(CCH, P).T),
        "t2": np.ascontiguousarray(t2.reshape(CCH, P).T),
        "s1": np.ascontiguousarray(s1.reshape(K, 1)),
        "t1": np.ascontiguousarray(t1.reshape(K, 1)),
        "fcwT": np.ascontiguousarray(fc_w.T),
        "fcb": np.ascontiguousarray(fc_b.reshape(CCH, P).T),
        "ident": np.eye(P, dtype=f),
        "ones_col": np.ones((P, 1), f),
    }

    x = np.asarray(inputs["x"], f).reshape(B, C, N)
    in_maps = []
    for core in range(NCORES):
        b, h = core // 2, core % 2
        m = dict(common)
        m["x"] = np.ascontiguousarray(x[b, :, h * NSH:(h + 1) * NSH]).astype(bf)
        in_maps.append(m)
    return in_maps


def kernel(**inputs):
    _import_concourse()
    from concourse.bass_utils import run_bass_kernel_spmd

    nc = _build_program()
    in_maps = _host_prep(inputs)
    res = run_bass_kernel_spmd(nc, in_maps, list(range(NCORES)), trace=TRACE)
    _prog_cache["last_results"] = res

    out = np.empty((B, C, N), np.float32)
    feat = np.empty((B, C), np.float32)
    for core in range(NCORES):
        b, h = core // 2, core % 2
        out[b, :, h * NSH:(h + 1) * NSH] = res.results[core]["out"]
        if h == 0:
            feat[b] = res.results[core]["feat"].T.reshape(C)
    return feat, out.reshape(B, C, H, W)
